# revision 8
# baseline (speedup 1.0000x reference)
"""Trainium2 Bass kernel for nn_Box_Rel_Classifier.

Math (per output element, i over box2 rows, j over box1 rows, d over dims):
  z  = sigmoid(x0 - softplus(10*x1)/10),  Z = sigmoid(x0 + softplus(10*x1)/10)
  out_min[i*160+j, d] = gb*logsumexp(z2/gb, z1/gb)  ~= max(z2[i,d], z1[j,d])
  out_max[i*160+j, d] = -gb*logsumexp(-Z2/gb,-Z1/gb) ~= min(Z2[i,d], Z1[j,d])

With gb=0.0036 the log1p correction is <= gb*ln2 ~= 0.0025 absolute (~3e-4
relative in norm, far inside the 2e-2 gate), so it is dropped: the kernel
computes a plain pairwise max/min of 255-scaled sigmoids and stores uint8
(all convert paths round to nearest; the host divides by 255).  Total rel
err ~2e-3 with ~8x margin, and output DMA traffic drops 4x vs fp32.

Per-core schedule (box2 sharded 8 ways, 128 rows/core = the partition dim,
free axis = (j,d) = 40960 cols per tensor, processed as 20 units of 2048):
  PE  : psum = bcast of z1-table row (255*z1 as bf16 hi+lo, K=2 ones
        matmuls of 512 cols).
  R1  : DVE scalar_tensor_tensor: osb_u8 = max/min(psum, rep_f32); HWDGE out
  R2  : ACT copy psum->fp16; DVE tensor_tensor fp16 (2x mode) vs rep_f16;
        SWDGE (gpsimd) casting DMA fp16 -> u8 DRAM
  R1:R2 = 5:15 units per tensor so DVE and ACT both land at ~58us/core.
"""

import os
import sys

import numpy as np

try:
    import concourse.bacc as bacc  # noqa: F401
except ImportError:
    for p in ("/root/.axon_site/_ro/trn_rl_repo", "/opt/trn_rl_repo"):
        if p not in sys.path:
            sys.path.insert(0, p)
    import concourse.bacc as bacc

import concourse.bacc as bacc
import concourse.hw_specs as hw_specs
import concourse.tile as tile
from concourse import mybir
from concourse.bass_utils import run_bass_kernel_spmd

# ---- activation-table set selection patch ----------------------------------
# Keep prep's Abs/Exp/Ln in one table set and Sigmoid in another so the
# table-load inserter emits at most a couple of ACT_TABLE_LOADs.
_orig_gat = hw_specs.get_activation_tables


def _patched_gat(arch):
    tabs = _orig_gat(arch)
    hot = {
        mybir.ActivationFunctionType.Abs,
        mybir.ActivationFunctionType.Exp,
        mybir.ActivationFunctionType.Ln,
    }
    sig = {mybir.ActivationFunctionType.Sigmoid}
    out = {}
    for name, funcs in tabs.items():
        if name == "natural_log_exp_and_others":
            out[name] = funcs
        elif name == "sigmoid_and_others":
            out[name] = funcs - hot
        else:
            out[name] = funcs - hot - sig
    return out


bacc.get_activation_tables = _patched_gat

AF = mybir.ActivationFunctionType
ALU = mybir.AluOpType
F32 = mybir.dt.float32
BF16 = mybir.dt.bfloat16
F16 = mybir.dt.float16
U8 = mybir.dt.uint8

N1, N2, D = 160, 1024, 256
NCORES = 8
SH = N2 // NCORES          # 128 box2 rows per core
ROWS = SH * N1             # 20480 output rows per core
FLAT = N1 * D              # 40960 free columns per tensor
HALF = FLAT // 2           # 20480 (cols per tab row)
UNIT = 2048                # free cols per unit (8 j-rows)
NU = FLAT // UNIT          # 20 units per tensor

NR1 = int(os.environ.get("KERNEL_NR1", "5"))    # R1 (STT/u8) units per tensor
G16 = int(os.environ.get("KERNEL_G16", "3"))    # R2 units per casting DMA

_CACHE = {}


def _emit_z(nc, pool, x0, x1, p):
    """zmin/zmax pre-activations for p rows: returns (v, v2) with
    zmin = Sigmoid(-v), zmax = Sigmoid(v2)."""
    u1 = pool.tile([p, D], F32, tag=f"u1_{p}", name=f"u1_{p}")
    nc.scalar.activation(u1[:], x1[:], AF.Abs, scale=10.0)
    e1 = pool.tile([p, D], F32, tag=f"e1_{p}", name=f"e1_{p}")
    nc.scalar.activation(e1[:], u1[:], AF.Exp, scale=-1.0)
    l1 = pool.tile([p, D], F32, tag=f"l1_{p}", name=f"l1_{p}")
    nc.scalar.activation(l1[:], e1[:], AF.Ln, bias=1.0)
    q = pool.tile([p, D], F32, tag=f"q_{p}", name=f"q_{p}")
    nc.vector.scalar_tensor_tensor(out=q[:], in0=x1[:], scalar=0.0, in1=x0[:],
                                   op0=ALU.max, op1=ALU.subtract)
    v = pool.tile([p, D], F32, tag=f"v_{p}", name=f"v_{p}")
    nc.vector.scalar_tensor_tensor(out=v[:], in0=l1[:], scalar=0.1, in1=q[:],
                                   op0=ALU.mult, op1=ALU.add)
    q2 = pool.tile([p, D], F32, tag=f"q2_{p}", name=f"q2_{p}")
    nc.vector.scalar_tensor_tensor(out=q2[:], in0=x1[:], scalar=0.0, in1=x0[:],
                                   op0=ALU.max, op1=ALU.add)
    v2 = pool.tile([p, D], F32, tag=f"v2_{p}", name=f"v2_{p}")
    nc.vector.scalar_tensor_tensor(out=v2[:], in0=l1[:], scalar=0.1, in1=q2[:],
                                   op0=ALU.mult, op1=ALU.add)
    return v, v2


def _hi_lo(nc, pool, src, p, nm):
    """Split fp32 [p, D] into bf16 hi + bf16 lo (hi+lo ~= src to ~2^-18)."""
    hi = pool.tile([p, D], BF16, tag=f"{nm}hi", name=f"{nm}hi")
    nc.vector.tensor_copy(out=hi[:], in_=src[:])
    lo = pool.tile([p, D], BF16, tag=f"{nm}lo", name=f"{nm}lo")
    nc.vector.tensor_sub(lo[:], src[:], hi[:])
    return hi, lo


def _build():
    nc = bacc.Bacc("TRN2", target_bir_lowering=False, debug=False)

    box1 = nc.dram_tensor("box1s", [N1, 2, D], F32, kind="ExternalInput").ap()
    box2 = nc.dram_tensor("box2s", [SH, 2, D], F32, kind="ExternalInput").ap()
    outs = [
        nc.dram_tensor("out_min", [SH, N1, D], U8, kind="ExternalOutput").ap(),
        nc.dram_tensor("out_max", [SH, N1, D], U8, kind="ExternalOutput").ap(),
    ]

    with tile.TileContext(nc) as tc:
        with (
            tc.tile_pool(name="persist", bufs=1) as persist,
            tc.tile_pool(name="dram", bufs=1, space="DRAM") as dram,
            tc.tile_pool(name="work", bufs=4) as work,
            tc.tile_pool(name="outp", bufs=2) as outp,
            tc.tile_pool(name="psum", bufs=2, space="PSUM") as psum,
        ):
            # ---------------- constants ----------------
            w_ones = persist.tile([98, 128], BF16)
            nc.vector.memset(w_ones[:], 1.0)

            # rep tiles: 255*z2 replicated 8x along free (2048 = 1 unit)
            repf = [persist.tile([SH, UNIT], F32, tag=f"repf{t}",
                                 name=f"repf{t}")
                    for t in range(2)]
            reph = [persist.tile([SH, UNIT], F16, tag=f"reph{t}",
                                 name=f"reph{t}")
                    for t in range(2)]
            # bf16 table rows (hi on even row, lo on odd row of each pair)
            tab = persist.tile([98, HALF], BF16, tag="tab")
            zscr = dram.tile([4, N1, D], BF16)

            with tc.tile_pool(name="prep", bufs=1) as prep:
                # box2 shard
                x0_2 = prep.tile([SH, D], F32)
                nc.sync.dma_start(out=x0_2[:], in_=box2[:, 0, :])
                x1_2 = prep.tile([SH, D], F32)
                nc.sync.dma_start(out=x1_2[:], in_=box2[:, 1, :])
                v2min, v2max = _emit_z(nc, prep, x0_2, x1_2, SH)

                # box1 table (two partition chunks)
                x0_a = prep.tile([128, D], F32, tag="x0_a")
                nc.sync.dma_start(out=x0_a[:], in_=box1[0:128, 0, :])
                x1_a = prep.tile([128, D], F32, tag="x1_a")
                nc.sync.dma_start(out=x1_a[:], in_=box1[0:128, 1, :])
                va_min, va_max = _emit_z(nc, prep, x0_a, x1_a, 128)

                x0_b = prep.tile([32, D], F32, tag="x0_b")
                nc.sync.dma_start(out=x0_b[:], in_=box1[128:160, 0, :])
                x1_b = prep.tile([32, D], F32, tag="x1_b")
                nc.sync.dma_start(out=x1_b[:], in_=box1[128:160, 1, :])
                vb_min, vb_max = _emit_z(nc, prep, x0_b, x1_b, 32)

                # sigmoids scaled by 255 on the way out is not possible in
                # one ACT op (scale applies to the input), so: sigmoid then
                # scale via ACT Copy.
                def sig_scaled(v, p, nm, negate):
                    s = prep.tile([p, D], F32, tag=f"s{nm}", name=f"s{nm}")
                    nc.scalar.activation(s[:], v[:], AF.Sigmoid,
                                         scale=-1.0 if negate else 1.0)
                    t = prep.tile([p, D], F32, tag=f"t{nm}", name=f"t{nm}")
                    nc.scalar.activation(t[:], s[:], AF.Copy, scale=255.0)
                    return t

                t2 = [sig_scaled(v2min, SH, "t2min", True),
                      sig_scaled(v2max, SH, "t2max", False)]
                t1a = [sig_scaled(va_min, 128, "t1amin", True),
                       sig_scaled(va_max, 128, "t1amax", False)]
                t1b = [sig_scaled(vb_min, 32, "t1bmin", True),
                       sig_scaled(vb_max, 32, "t1bmax", False)]

                # box2 reps (8 copies of D)
                for t in range(2):
                    for k in range(UNIT // D):
                        nc.vector.tensor_copy(
                            out=repf[t][:, k * D:(k + 1) * D], in_=t2[t][:])
                    nc.vector.tensor_copy(out=reph[t][:], in_=repf[t][:])

                # box1 tables: bf16 hi/lo -> DRAM -> flat tab rows
                for t, nm in ((0, "z"), (1, "Z")):
                    ah, al = _hi_lo(nc, prep, t1a[t], 128, f"{nm}a")
                    bh, bl = _hi_lo(nc, prep, t1b[t], 32, f"{nm}b")
                    nc.sync.dma_start(out=zscr[2 * t, 0:128, :], in_=ah[:])
                    nc.sync.dma_start(out=zscr[2 * t, 128:160, :], in_=bh[:])
                    nc.sync.dma_start(out=zscr[2 * t + 1, 0:128, :], in_=al[:])
                    nc.sync.dma_start(out=zscr[2 * t + 1, 128:160, :],
                                      in_=bl[:])

                # tab rows: base+0 = hi, base+1 = lo
                # z1 halves 0/1 -> rows 0/1, 32/33; Z1 -> rows 64/65, 96/97
                for src, r0 in [(0, 0), (2, 64)]:
                    for half, radd in [(0, 0), (1, 32)]:
                        rows = slice(half * 80, half * 80 + 80)
                        nc.sync.dma_start(
                            out=tab[r0 + radd:r0 + radd + 1, :],
                            in_=zscr[src, rows, :]
                            .rearrange("(o r) d -> o (r d)", o=1))
                        nc.sync.dma_start(
                            out=tab[r0 + radd + 1:r0 + radd + 2, :],
                            in_=zscr[src + 1, rows, :]
                            .rearrange("(o r) d -> o (r d)", o=1))

            # ---------------- main loop ----------------
            # tensor t: 0 = out_min (op max), 1 = out_max (op min)
            ops = [ALU.max, ALU.min]
            UJ = UNIT // D        # j rows per unit (8)

            def mm(p, t, u):
                h = 0 if u < NU // 2 else 1
                off = (u % (NU // 2)) * UNIT
                prow = 64 * t + 32 * h
                for c in range(UNIT // 512):
                    nc.tensor.matmul(
                        p[:, c * 512:(c + 1) * 512],
                        lhsT=w_ones[prow:prow + 2, :],
                        rhs=tab[prow:prow + 2,
                                off + c * 512:off + c * 512 + 512],
                        start=True, stop=True, tile_position=(prow, 0))

            def emit_r1(t, u):
                p = psum.tile([128, UNIT], F32, tag="ps", name=f"ps1_{t}_{u}")
                mm(p, t, u)
                osb = outp.tile([128, UNIT], U8, tag=f"o8_{t}",
                                name=f"o8_{t}_{u}")
                nc.vector.scalar_tensor_tensor(
                    out=osb[:], in0=p[:], scalar=0.0, in1=repf[t][:],
                    op0=ALU.bypass, op1=ops[t])
                j0 = u * UJ
                nc.sync.dma_start(
                    out=outs[t][:, j0:j0 + UJ, :],
                    in_=osb.rearrange("p (r d) -> p r d", d=D))

            def emit_r2_group(t, us):
                osb = outp.tile([128, len(us) * UNIT], F16, tag=f"o16_{t}",
                                name=f"o16_{t}_{us[0]}")
                for k, u in enumerate(us):
                    p = psum.tile([128, UNIT], F32, tag="ps",
                                  name=f"ps2_{t}_{u}")
                    mm(p, t, u)
                    zh = work.tile([128, UNIT], F16, tag="zh",
                                   name=f"zh_{t}_{u}")
                    nc.scalar.activation(zh[:], p[:], AF.Copy)
                    nc.vector.tensor_tensor(
                        out=osb[:, k * UNIT:(k + 1) * UNIT],
                        in0=zh[:], in1=reph[t][:], op=ops[t])
                j0 = us[0] * UJ
                nc.gpsimd.dma_start(
                    out=outs[t][:, j0:j0 + len(us) * UJ, :],
                    in_=osb.rearrange("p (r d) -> p r d", d=D))

            # Interleave R1 units (j < NR1*UJ) between R2 groups so DVE and
            # ACT stay concurrently busy from the start.
            nr2 = NU - NR1
            r2_units = list(range(NR1, NU))
            r2_groups = [r2_units[i:i + G16] for i in range(0, nr2, G16)]
            seq = []
            for k in range(max(len(r2_groups), NR1)):
                if k < len(r2_groups):
                    seq.append(("r2", r2_groups[k]))
                if k < NR1:
                    seq.append(("r1", k))
            for kind, arg in seq:
                for t in range(2):
                    if kind == "r1":
                        emit_r1(t, arg)
                    else:
                        emit_r2_group(t, arg)

    nc.compile()
    return nc


def _get_nc():
    if "nc" not in _CACHE:
        _CACHE["nc"] = _build()
    return _CACHE["nc"]


def make_in_maps(box1s, box2s):
    box1s = np.ascontiguousarray(np.asarray(box1s, dtype=np.float32))
    box2s = np.ascontiguousarray(np.asarray(box2s, dtype=np.float32))
    return [
        {
            "box1s": box1s,
            "box2s": np.ascontiguousarray(box2s[c * SH:(c + 1) * SH]),
        }
        for c in range(NCORES)
    ]


def kernel(box1s, box2s):
    nc = _get_nc()
    res = run_bass_kernel_spmd(nc, make_in_maps(box1s, box2s),
                               core_ids=list(range(NCORES)))
    inv = np.float32(1.0) / np.float32(255.0)
    out_min = np.concatenate(
        [r["out_min"].reshape(SH * N1, D) for r in res.results],
        axis=0).astype(np.float32) * inv
    out_max = np.concatenate(
        [r["out_max"].reshape(SH * N1, D) for r in res.results],
        axis=0).astype(np.float32) * inv
    return out_min, out_max
